# revision 6
# baseline (speedup 1.0000x reference)
"""Trainium2 Bass kernel for nn_BlockBlastValueNet1PmultikernelFlattenned.

Strategy (v2)
-------------
The network is 8 tiny conv branches over an 8x8 board followed by small MLPs.
Because the board has only 64 pixels, every conv branch (pad const 1.0 +
valid conv + bias) is an affine map of the 64 board values.  The whole net
folds into:

    y  = x @ W1 + c1                     # [B, NF]  (NF = 2944 padded)
    h  = Lrelu( Lrelu(y) @ W2' + b2 )    # per-branch first FC, block diagonal
    g1 = Lrelu( h @ W3 + b3 )            # branch second FC fused with fc1
    g2 = Lrelu( g1 @ W4 + b4 )           # fc2 (augmented with a ones column)
    out = g2 @ W5                        # fc3 (bias folded via augmentation)

Data-parallel over 8 NeuronCores (batch 65536 -> 8192/core), processed in
8 pair-iterations of 1024 samples.  Features live on SBUF partitions,
samples stream on the free dim.  Matmuls run in fp16 (full PE rate, warm
2.4 GHz clock).

The binding resource is PSUM evacuation: every y element must pass through
the Scalar or Vector engine exactly once (1 elem/cycle/lane each), so the
whole schedule is built to keep both engines 100% busy while the PE (which
has slack) works around them:

* Step-1 (x -> y) is row-packed: two 64x128 weight tiles at tile_position
  (0,0)/(64,0) run concurrently, 12 slots per pair.
* y evacuation is split ~12/11 between Vector (tensor_scalar add+max =
  relu(y+c1), with the 0.01*y leaky path folded into a small 64->128
  "fold" matmul W12 covering only the vector-assigned tiles) and Scalar
  (exact Lrelu via ACT with per-partition bias — no decomposition needed).
* Step-2 (y -> h) exploits W2's block-diagonal structure with PE column
  tiling: branches are paired into four 32-column groups; each group is an
  accumulation chain over only ITS y tiles at tile_position (0, 32g), so
  the four chains (9/6/5/6 tiles) run concurrently in the four column
  quadrants.  This removes the old row-half partial-sum merge entirely
  (saving two 128x1024 Vector ops per pair) and halves step-2 PE time.
* The serial tail (h -> g1 -> g2 -> out) runs as fp16 matmuls + Scalar
  ACTs, interleaved two slots apart into the NEXT pair's slot stream so
  its cross-engine latencies never head-of-line-block the PE queue.
* ps2 (the h accumulator) is written first by the fold matmul at slot 2,
  after the previous pair's h evacuation has had time to finish (avoids
  the PE queue parking on the PSUM WAR dependency).
* Startup weight DMAs are spread over the gpsimd/vector/scalar queues so
  the first matmul starts ~3us in instead of ~14us.
"""

import numpy as np

# ---------------------------------------------------------------- constants
SPECS = [(1, 1, 1, 0, 0), (2, 2, 6, 1, 1), (3, 3, 8, 1, 1), (4, 4, 8, 2, 2),
         (5, 5, 16, 2, 2), (8, 8, 32, 0, 0), (1, 8, 4, 0, 0), (8, 1, 4, 0, 0)]
BOARD = 8
B_TOTAL = 65536
N_CORES = 8
BC = B_TOTAL // N_CORES          # 8192 samples per core
PAIR_N = 1024                    # samples per pair-iteration
CHUNK = 512                      # matmul moving width (1 psum bank fp32)
N_PAIRS = BC // PAIR_N           # 8

# column-group pairing of branches (2 branches x 16 h-outputs = 32 cols each).
# y rows are packed tightly in group order; a K-tile straddling a group
# boundary appears in BOTH groups' chains (the block-diagonal W2 already has
# zeros for the other group's columns).
GROUPS = [[4, 5], [3, 6], [2, 7], [1, 0]]
_BR_N = []
for kh, kw, fs, ph, pw in SPECS:
    _BR_N.append((BOARD + 2 * ph - kh + 1) * (BOARD + 2 * pw - kw + 1) * fs)
_NF_TRUE = sum(_BR_N)            # 2830
KT = -(-_NF_TRUE // 128)         # 23 K-tiles / M-tiles of y
NF = KT * 128                    # 2944 (last tile zero-padded)
_G_ROWS = []
_o = 0
for g in GROUPS:
    sz = sum(_BR_N[b] for b in g)
    _G_ROWS.append((_o, _o + sz))
    _o += sz
GROUP_TR = [(r0 // 128, -(-r1 // 128)) for r0, r1 in _G_ROWS]  # tile ranges
N_S1 = (KT + 1) // 2             # step-1 row-packed slots (12)

LRELU_NEG = 0.01

# evac engine split: these tiles go to the Vector engine (relu + fold path),
# the rest to Scalar (exact Lrelu ACT).  Slots 0/1 are all-vector because at
# pair start the Scalar engine is still busy with the previous pair's tail.
VEC_TILES = frozenset({0, 1, 2, 3, 5, 7, 9, 11, 13, 15, 17, 19})


# ---------------------------------------------------------------- host fold
def _fold_params(p):
    """Fold conv branches + MLPs into the dense pipeline weights (float64)."""
    n_of = _BR_N
    W1_of, c1_of = {}, {}
    for i, (kh, kw, fs, ph, pw) in enumerate(SPECS):
        Ho = BOARD + 2 * ph - kh + 1
        Wo = BOARD + 2 * pw - kw + 1
        cw = np.asarray(p[f"b{i}_cw"], np.float64)
        cb = np.asarray(p[f"b{i}_cb"], np.float64)
        W1 = np.zeros((64, n_of[i]))
        c1 = np.zeros((n_of[i],))
        for f in range(fs):
            for oh in range(Ho):
                for ow in range(Wo):
                    oi = (f * Ho + oh) * Wo + ow
                    c1[oi] += cb[f]
                    for u in range(kh):
                        for v in range(kw):
                            r, c = oh + u - ph, ow + v - pw
                            w = cw[f, 0, u, v]
                            if 0 <= r < 8 and 0 <= c < 8:
                                W1[r * 8 + c, oi] += w
                            else:
                                c1[oi] += w        # pad value is 1.0
        W1_of[i] = W1
        c1_of[i] = c1

    # K-layout: groups concatenated tightly; only the final tile is padded
    K_start = {}
    off = 0
    for g in GROUPS:
        for b in g:
            K_start[b] = off
            off += n_of[b]
    assert off == _NF_TRUE
    border = [b for g in GROUPS for b in g]       # h block order
    hpos = {b: j * 16 for j, b in enumerate(border)}

    W1p = np.zeros((64, NF))
    c1p = np.zeros((NF,))
    W2p = np.zeros((NF, 128))
    b2p = np.zeros((128,))
    for b in range(8):
        s, n, hp = K_start[b], n_of[b], hpos[b]
        W1p[:, s:s + n] = W1_of[b]
        c1p[s:s + n] = c1_of[b]
        W2p[s:s + n, hp:hp + 16] = np.asarray(p[f"b{b}_w1"], np.float64).T
        b2p[hp:hp + 16] = np.asarray(p[f"b{b}_b1"], np.float64)

    Wb = np.zeros((128, 64))
    bb = np.zeros((64,))
    for b in range(8):
        hp = hpos[b]
        Wb[hp:hp + 16, 8 * b:8 * b + 8] = np.asarray(p[f"b{b}_w2"], np.float64).T
        bb[8 * b:8 * b + 8] = np.asarray(p[f"b{b}_b2"], np.float64)
    fc_w1 = np.asarray(p["fc_w1"], np.float64)
    fc_b1 = np.asarray(p["fc_b1"], np.float64)
    W3 = Wb @ fc_w1.T
    b3 = bb @ fc_w1.T + fc_b1
    fc_w2 = np.asarray(p["fc_w2"], np.float64)
    fc_b2 = np.asarray(p["fc_b2"], np.float64)
    fc_w3 = np.asarray(p["fc_w3"], np.float64)
    fc_b3 = np.asarray(p["fc_b3"], np.float64)
    W4 = np.zeros((64, 17)); W4[:, :16] = fc_w2.T
    b4 = np.zeros((17,)); b4[:16] = fc_b2; b4[16] = 1.0
    W5 = np.zeros((17, 1)); W5[:16, 0] = fc_w3[0]; W5[16, 0] = fc_b3[0]

    # Vector-assigned tiles use relu + decomposition:
    #   Lrelu(v) = 0.01*v + 0.99*relu(v)
    # so their W2 rows are scaled by 0.99 and the 0.01*v path (linear in x)
    # folds into W12 / b2f.  Scalar-assigned tiles compute Lrelu exactly on
    # the ACT engine, so their W2 rows stay unscaled and contribute nothing
    # to the fold.
    vmask = np.zeros((NF, 1))
    for t in range(KT):
        if t in VEC_TILES:
            vmask[128 * t:128 * (t + 1)] = 1.0
    W2s = np.where(vmask > 0, (1.0 - LRELU_NEG) * W2p, W2p)
    W12 = LRELU_NEG * ((W1p * vmask[:, 0][None, :]) @ W2p)
    b2f = LRELU_NEG * ((c1p * vmask[:, 0]) @ W2p) + b2p

    f32 = np.float32
    f16 = np.float16
    dev = {}
    # step-1 weights packed for row-pairing: slot s holds M-tiles 2s | 2s+1
    w1 = np.zeros((128, N_S1, 128), f16)
    for s in range(N_S1):
        w1[0:64, s, :] = W1p[:, 128 * (2 * s):128 * (2 * s + 1)]
        if 2 * s + 1 < KT:
            w1[64:128, s, :] = W1p[:, 128 * (2 * s + 1):128 * (2 * s + 2)]
    dev["w1"] = w1
    c1t = np.zeros((128, KT), f32)
    for t in range(KT):
        c1t[:, t] = c1p[128 * t:128 * (t + 1)]
    dev["c1t"] = c1t
    w2 = np.zeros((128, KT, 128), f16)
    for t in range(KT):
        w2[:, t, :] = W2s[128 * t:128 * (t + 1), :]
    dev["w2"] = w2
    dev["w12"] = W12.astype(f16)
    dev["b2f"] = b2f.reshape(128, 1).astype(f32)
    dev["w3"] = W3.astype(f16)
    dev["b3"] = b3.reshape(64, 1).astype(f32)
    dev["w4"] = W4.astype(f16)
    dev["b4"] = b4.reshape(17, 1).astype(f32)
    dev["w5"] = W5.astype(f16)
    return dev


# ---------------------------------------------------------------- device IR
def _build_nc(n_pairs=N_PAIRS):
    import concourse.mybir as mybir
    import concourse.tile as tile
    from concourse import bacc
    from contextlib import ExitStack

    dt = mybir.dt
    AF = mybir.ActivationFunctionType
    ALU = mybir.AluOpType
    f32 = dt.float32
    f16 = dt.float16
    bc = n_pairs * PAIR_N

    nc = bacc.Bacc("TRN2", target_bir_lowering=False, debug=False,
                   num_devices=N_CORES)

    xx_d = nc.dram_tensor("xx", [128, bc], f16, kind="ExternalInput")
    w1_d = nc.dram_tensor("w1", [128, N_S1, 128], f16, kind="ExternalInput")
    c1t_d = nc.dram_tensor("c1t", [128, KT], f32, kind="ExternalInput")
    w2_d = nc.dram_tensor("w2", [128, KT, 128], f16, kind="ExternalInput")
    w12_d = nc.dram_tensor("w12", [64, 128], f16, kind="ExternalInput")
    b2f_d = nc.dram_tensor("b2f", [128, 1], f32, kind="ExternalInput")
    w3_d = nc.dram_tensor("w3", [128, 64], f16, kind="ExternalInput")
    b3_d = nc.dram_tensor("b3", [64, 1], f32, kind="ExternalInput")
    w4_d = nc.dram_tensor("w4", [64, 17], f16, kind="ExternalInput")
    b4_d = nc.dram_tensor("b4", [17, 1], f32, kind="ExternalInput")
    w5_d = nc.dram_tensor("w5", [17, 1], f16, kind="ExternalInput")
    o_d = nc.dram_tensor("o", [1, bc], f32, kind="ExternalOutput")

    with tile.TileContext(nc) as tc, ExitStack() as ctx:
        wpool = ctx.enter_context(tc.tile_pool(name="wpool", bufs=1))
        xpool = ctx.enter_context(tc.tile_pool(name="xpool", bufs=3))
        ypool = ctx.enter_context(tc.tile_pool(name="ypool", bufs=KT + 3))
        spool = ctx.enter_context(tc.tile_pool(name="spool", bufs=2))
        ps1p = ctx.enter_context(tc.tile_pool(name="ps1p", bufs=3, space="PSUM"))
        ps2p = ctx.enter_context(tc.tile_pool(name="ps2p", bufs=1, space="PSUM"))

        # pair-0 input first so compute can start while the rest streams in
        xx_first = xpool.tile([128, PAIR_N], f16, tag="xx", name="xx_first")
        nc.sync.dma_start(xx_first[:], xx_d[:, 0:PAIR_N])
        # weight DMAs spread across engine queues: gpsimd carries the
        # step-1-critical tensors, vector the (later-needed) step-2 weights,
        # scalar the fold + tail weights.
        w1_t = wpool.tile([128, N_S1, 128], f16)
        nc.gpsimd.dma_start(w1_t[:], w1_d[:])
        c1t_t = wpool.tile([128, KT], f32)
        nc.gpsimd.dma_start(c1t_t[:], c1t_d[:])
        b2f_t = wpool.tile([128, 1], f32)
        nc.gpsimd.dma_start(b2f_t[:], b2f_d[:])
        w2_t = wpool.tile([128, KT, 128], f16)
        nc.scalar.dma_start(w2_t[:], w2_d[:])
        w12_t = wpool.tile([64, 128], f16)
        nc.gpsimd.dma_start(w12_t[:], w12_d[:])
        w3_t = wpool.tile([128, 64], f16)
        nc.gpsimd.dma_start(w3_t[:], w3_d[:])
        b3_t = wpool.tile([64, 1], f32)
        nc.gpsimd.dma_start(b3_t[:], b3_d[:])
        w4_t = wpool.tile([64, 17], f16)
        nc.gpsimd.dma_start(w4_t[:], w4_d[:])
        b4_t = wpool.tile([17, 1], f32)
        nc.gpsimd.dma_start(b4_t[:], b4_d[:])
        w5_t = wpool.tile([17, 1], f16)
        nc.gpsimd.dma_start(w5_t[:], w5_d[:])

        # PE warm-up: ~4us of dummy back-to-back matmuls on a zeroed tile so
        # the HAM clock-gate opens (1.2 -> 2.4 GHz) during the startup DMA
        # window instead of half-way through pair 1.
        wu_t = wpool.tile([64, 640], f16, name="wu")
        nc.vector.memset(wu_t[:], 0.0)
        wups = ps1p.tile([128, CHUNK], f32, tag="ps1", name="wups")
        for i in range(9):
            nc.tensor.matmul(wups[:], wu_t[:, 0:128], wu_t[:, 128:640],
                             start=True, stop=True)

        def make_tail_stages(p, ps2):
            """The per-pair serial tail (h -> g1 -> g2 -> out).  The h ACT
            pops at slot 0 of the next pair (it is the Scalar queue's first
            op there); the matmul + later ACT stages run as fillers inside
            the next pair's chain phase, where the PE is already
            row-conflicted, so they never poison the pure slot stream."""
            st = {}

            def s0():
                st["h"] = spool.tile([128, PAIR_N], f16, tag="h", name=f"h_{p}")
                nc.scalar.activation(st["h"][:], ps2[:], AF.Lrelu,
                                     bias=b2f_t[:, 0:1], alpha=LRELU_NEG)

            def s1():
                st["g1ps"] = ps1p.tile([64, PAIR_N], f32, tag="ps1",
                                       name=f"g1ps_{p}")
                for h in range(2):
                    sl = slice(h * CHUNK, (h + 1) * CHUNK)
                    nc.tensor.matmul(st["g1ps"][:, sl], w3_t[:],
                                     st["h"][:, sl], start=True, stop=True)

            def s2():
                st["g1"] = spool.tile([64, PAIR_N], f16, tag="g1",
                                      name=f"g1_{p}")
                nc.scalar.activation(st["g1"][:], st["g1ps"][:], AF.Lrelu,
                                     bias=b3_t[:, 0:1], alpha=LRELU_NEG)

            def s3():
                st["g2ps"] = ps1p.tile([17, PAIR_N], f32, tag="ps1",
                                       name=f"g2ps_{p}")
                for h in range(2):
                    sl = slice(h * CHUNK, (h + 1) * CHUNK)
                    nc.tensor.matmul(st["g2ps"][:, sl], w4_t[:],
                                     st["g1"][:, sl], start=True, stop=True)

            def s4():
                st["g2"] = spool.tile([17, PAIR_N], f16, tag="g2",
                                      name=f"g2_{p}")
                nc.scalar.activation(st["g2"][:], st["g2ps"][:], AF.Lrelu,
                                     bias=b4_t[:, 0:1], alpha=LRELU_NEG)

            def s5():
                st["ops"] = ps1p.tile([1, PAIR_N], f32, tag="ps1",
                                      name=f"ops_{p}")
                for h in range(2):
                    sl = slice(h * CHUNK, (h + 1) * CHUNK)
                    nc.tensor.matmul(st["ops"][:, sl], w5_t[:],
                                     st["g2"][:, sl], start=True, stop=True)

            def s6():
                o_t = spool.tile([1, PAIR_N], f32, tag="o", name=f"o_{p}")
                nc.vector.tensor_copy(o_t[:], st["ops"][:])
                nc.sync.dma_start(o_d[:, p * PAIR_N:(p + 1) * PAIR_N], o_t[:])

            return [s0, s1, s2, s3, s4, s5, s6]

        tail_stages = []

        xx_next = xx_first
        for p in range(n_pairs):
            xx_t = xx_next
            stages = tail_stages
            tail_stages = []

            ps2 = ps2p.tile([128, PAIR_N], f32, tag="ps2", name=f"ps2_{p}")
            ytiles = [None] * KT

            # ---- slot phase: pure [A0, B0, A1, B1] stream + evacuations.
            # LDWEIGHTS for slot s+1 hides under slot s's streaming because
            # nothing with conflicting row-groups sits between slots.
            for s in range(N_S1):
                if s == 0 and stages:
                    stages[0]()          # h-act of prev pair (Scalar op #1)
                if p + 1 < n_pairs and s == 1:
                    xx_next = xpool.tile([128, PAIR_N], f16, tag="xx",
                                         name=f"xx_{p + 1}")
                    nc.sync.dma_start(
                        xx_next[:],
                        xx_d[:, (p + 1) * PAIR_N:(p + 2) * PAIR_N])

                tA, tB = 2 * s, 2 * s + 1
                psA = ps1p.tile([128, PAIR_N], f32, tag="ps1",
                                name=f"psA_{p}_{s}")
                psB = None
                if tB < KT:
                    psB = ps1p.tile([128, PAIR_N], f32, tag="ps1",
                                    name=f"psB_{p}_{s}")
                # Each chunk is emitted twice: the first is a pure HAM-warmth
                # filler (the second start=True overwrites it, so results are
                # identical).  The slot phase is evacuation-paced with ~40%
                # PE idle; without the fillers the idle gaps re-throttle the
                # PE clock to 1.2 GHz every single pair (measured K=4/8 for
                # 17us of every 27us pair), which doubles the real matmuls'
                # duration.  Padding the stream keeps K=8/8 throughout.
                for h in range(2):
                    sl = slice(h * CHUNK, (h + 1) * CHUNK)
                    for _dup in range(2):
                        nc.tensor.matmul(
                            psA[:, sl], w1_t[0:64, s, :], xx_t[0:64, sl],
                            start=True, stop=True, tile_position=(0, 0))
                        if psB is not None:
                            nc.tensor.matmul(
                                psB[:, sl], w1_t[64:128, s, :],
                                xx_t[64:128, sl],
                                start=True, stop=True, tile_position=(64, 0))
                for t, ps in ((tA, psA),) + (((tB, psB),) if psB is not None
                                             else ()):
                    y_t = ypool.tile([128, PAIR_N], f16, tag="y",
                                     name=f"y_{p}_{t}")
                    if t in VEC_TILES:
                        nc.vector.tensor_scalar(
                            y_t[:], ps[:], c1t_t[:, t:t + 1], 0.0,
                            ALU.add, ALU.max)
                    else:
                        nc.scalar.activation(
                            y_t[:], ps[:], AF.Lrelu, bias=c1t_t[:, t:t + 1],
                            alpha=LRELU_NEG)
                    ytiles[t] = y_t

            # ---- chain phase: fold + four concurrent column chains,
            # chunk-major so the four 32-column tile positions stream
            # simultaneously; the previous pair's tail stages are used as
            # fillers between rounds.
            for h in range(2):
                sl = slice(h * CHUNK, (h + 1) * CHUNK)
                nc.tensor.matmul(
                    ps2[:, sl], w12_t[:], xx_t[0:64, sl],
                    start=True, stop=False, tile_position=(0, 0),
                    skip_group_check=True)

            chain_len = [t1 - t0 for t0, t1 in GROUP_TR]
            fillers = stages[1:] if stages else []
            fill_i = 0
            for r in range(max(chain_len)):
                for h in range(2):
                    sl = slice(h * CHUNK, (h + 1) * CHUNK)
                    for g in range(4):
                        if r >= chain_len[g]:
                            continue
                        t = GROUP_TR[g][0] + r
                        nc.tensor.matmul(
                            ps2[32 * g:32 * (g + 1), sl],
                            w2_t[:, t, 32 * g:32 * (g + 1)],
                            ytiles[t][:, sl], start=False,
                            stop=(r == chain_len[g] - 1),
                            tile_position=(0, 32 * g), skip_group_check=True)
                if fill_i < len(fillers):
                    fillers[fill_i]()
                    fill_i += 1
            while fill_i < len(fillers):
                fillers[fill_i]()
                fill_i += 1

            tail_stages = make_tail_stages(p, ps2)

        for st in tail_stages:
            st()

    nc.compile()
    return nc


# ---------------------------------------------------------------- execution
_NC_CACHE = {}
LAST_RESULT = None


def _prep_inputs(inputs):
    board = np.ascontiguousarray(np.asarray(inputs["board"], np.float32))
    x = board.reshape(B_TOTAL, 64)
    dev = _fold_params(inputs)
    in_maps = []
    for c in range(N_CORES):
        xc = np.ascontiguousarray(x[c * BC:(c + 1) * BC].T)      # [64, BC]
        m = dict(dev)
        m["xx"] = np.ascontiguousarray(
            np.vstack([xc, xc]).astype(np.float16))              # [128, BC]
        in_maps.append(m)
    return in_maps


def kernel(**inputs):
    global LAST_RESULT
    from concourse.bass_utils import run_bass_kernel_spmd

    if "nc" not in _NC_CACHE:
        _NC_CACHE["nc"] = _build_nc()
    nc = _NC_CACHE["nc"]

    in_maps = _prep_inputs(inputs)
    res = run_bass_kernel_spmd(nc, in_maps, core_ids=list(range(N_CORES)))
    LAST_RESULT = res
    out = np.concatenate([r["o"].reshape(-1) for r in res.results])
    return out.reshape(B_TOTAL, 1).astype(np.float32)


# revision 7
# speedup vs baseline: 1.1417x; 1.1417x over previous
"""Trainium2 Bass kernel for nn_BlockBlastValueNet1PmultikernelFlattenned.

Strategy (v2)
-------------
The network is 8 tiny conv branches over an 8x8 board followed by small MLPs.
Because the board has only 64 pixels, every conv branch (pad const 1.0 +
valid conv + bias) is an affine map of the 64 board values.  The whole net
folds into:

    y  = x @ W1 + c1                     # [B, NF]  (NF = 2944 padded)
    h  = Lrelu( Lrelu(y) @ W2' + b2 )    # per-branch first FC, block diagonal
    g1 = Lrelu( h @ W3 + b3 )            # branch second FC fused with fc1
    g2 = Lrelu( g1 @ W4 + b4 )           # fc2 (augmented with a ones column)
    out = g2 @ W5                        # fc3 (bias folded via augmentation)

Data-parallel over 8 NeuronCores (batch 65536 -> 8192/core), processed in
8 pair-iterations of 1024 samples.  Features live on SBUF partitions,
samples stream on the free dim.  Matmuls run in fp16 (full PE rate, warm
2.4 GHz clock).

The binding resource is PSUM evacuation: every y element must pass through
the Scalar or Vector engine exactly once (1 elem/cycle/lane each), so the
whole schedule is built to keep both engines 100% busy while the PE (which
has slack) works around them:

* Step-1 (x -> y) is row-packed: two 64x128 weight tiles at tile_position
  (0,0)/(64,0) run concurrently, 12 slots per pair.
* y evacuation is split ~12/11 between Vector (tensor_scalar add+max =
  relu(y+c1), with the 0.01*y leaky path folded into a small 64->128
  "fold" matmul W12 covering only the vector-assigned tiles) and Scalar
  (exact Lrelu via ACT with per-partition bias — no decomposition needed).
* Step-2 (y -> h) exploits W2's block-diagonal structure with PE column
  tiling: branches are paired into four 32-column groups; each group is an
  accumulation chain over only ITS y tiles at tile_position (0, 32g), so
  the four chains (9/6/5/6 tiles) run concurrently in the four column
  quadrants.  This removes the old row-half partial-sum merge entirely
  (saving two 128x1024 Vector ops per pair) and halves step-2 PE time.
* The serial tail (h -> g1 -> g2 -> out) runs as fp16 matmuls + Scalar
  ACTs, interleaved two slots apart into the NEXT pair's slot stream so
  its cross-engine latencies never head-of-line-block the PE queue.
* ps2 (the h accumulator) is written first by the fold matmul at slot 2,
  after the previous pair's h evacuation has had time to finish (avoids
  the PE queue parking on the PSUM WAR dependency).
* Startup weight DMAs are spread over the gpsimd/vector/scalar queues so
  the first matmul starts ~3us in instead of ~14us.
"""

import numpy as np

# ---------------------------------------------------------------- constants
SPECS = [(1, 1, 1, 0, 0), (2, 2, 6, 1, 1), (3, 3, 8, 1, 1), (4, 4, 8, 2, 2),
         (5, 5, 16, 2, 2), (8, 8, 32, 0, 0), (1, 8, 4, 0, 0), (8, 1, 4, 0, 0)]
BOARD = 8
B_TOTAL = 65536
N_CORES = 8
BC = B_TOTAL // N_CORES          # 8192 samples per core
PAIR_N = 1024                    # samples per pair-iteration
CHUNK = 512                      # matmul moving width (1 psum bank fp32)
N_PAIRS = BC // PAIR_N           # 8

# column-group pairing of branches (2 branches x 16 h-outputs = 32 cols each).
# y rows are packed tightly in group order; a K-tile straddling a group
# boundary appears in BOTH groups' chains (the block-diagonal W2 already has
# zeros for the other group's columns).
GROUPS = [[4, 5], [3, 6], [2, 7], [1, 0]]
_BR_N = []
for kh, kw, fs, ph, pw in SPECS:
    _BR_N.append((BOARD + 2 * ph - kh + 1) * (BOARD + 2 * pw - kw + 1) * fs)
_NF_TRUE = sum(_BR_N)            # 2830
KT = -(-_NF_TRUE // 128)         # 23 K-tiles / M-tiles of y
NF = KT * 128                    # 2944 (last tile zero-padded)
_G_ROWS = []
_o = 0
for g in GROUPS:
    sz = sum(_BR_N[b] for b in g)
    _G_ROWS.append((_o, _o + sz))
    _o += sz
GROUP_TR = [(r0 // 128, -(-r1 // 128)) for r0, r1 in _G_ROWS]  # tile ranges
N_S1 = (KT + 1) // 2             # step-1 row-packed slots (12)

LRELU_NEG = 0.01

# evac engine split: these tiles go to the Vector engine (relu + fold path),
# the rest to Scalar (exact Lrelu ACT).  Slots 0/1 are all-vector because at
# pair start the Scalar engine is still busy with the previous pair's tail.
VEC_TILES = frozenset({0, 1, 2, 3, 5, 7, 9, 11, 13, 15, 17, 19})


# ---------------------------------------------------------------- host fold
def _fold_params(p):
    """Fold conv branches + MLPs into the dense pipeline weights (float64)."""
    n_of = _BR_N
    W1_of, c1_of = {}, {}
    for i, (kh, kw, fs, ph, pw) in enumerate(SPECS):
        Ho = BOARD + 2 * ph - kh + 1
        Wo = BOARD + 2 * pw - kw + 1
        cw = np.asarray(p[f"b{i}_cw"], np.float64)
        cb = np.asarray(p[f"b{i}_cb"], np.float64)
        W1 = np.zeros((64, n_of[i]))
        c1 = np.zeros((n_of[i],))
        for f in range(fs):
            for oh in range(Ho):
                for ow in range(Wo):
                    oi = (f * Ho + oh) * Wo + ow
                    c1[oi] += cb[f]
                    for u in range(kh):
                        for v in range(kw):
                            r, c = oh + u - ph, ow + v - pw
                            w = cw[f, 0, u, v]
                            if 0 <= r < 8 and 0 <= c < 8:
                                W1[r * 8 + c, oi] += w
                            else:
                                c1[oi] += w        # pad value is 1.0
        W1_of[i] = W1
        c1_of[i] = c1

    # K-layout: groups concatenated tightly; only the final tile is padded
    K_start = {}
    off = 0
    for g in GROUPS:
        for b in g:
            K_start[b] = off
            off += n_of[b]
    assert off == _NF_TRUE
    border = [b for g in GROUPS for b in g]       # h block order
    hpos = {b: j * 16 for j, b in enumerate(border)}

    W1p = np.zeros((64, NF))
    c1p = np.zeros((NF,))
    W2p = np.zeros((NF, 128))
    b2p = np.zeros((128,))
    for b in range(8):
        s, n, hp = K_start[b], n_of[b], hpos[b]
        W1p[:, s:s + n] = W1_of[b]
        c1p[s:s + n] = c1_of[b]
        W2p[s:s + n, hp:hp + 16] = np.asarray(p[f"b{b}_w1"], np.float64).T
        b2p[hp:hp + 16] = np.asarray(p[f"b{b}_b1"], np.float64)

    Wb = np.zeros((128, 64))
    bb = np.zeros((64,))
    for b in range(8):
        hp = hpos[b]
        Wb[hp:hp + 16, 8 * b:8 * b + 8] = np.asarray(p[f"b{b}_w2"], np.float64).T
        bb[8 * b:8 * b + 8] = np.asarray(p[f"b{b}_b2"], np.float64)
    fc_w1 = np.asarray(p["fc_w1"], np.float64)
    fc_b1 = np.asarray(p["fc_b1"], np.float64)
    W3 = Wb @ fc_w1.T
    b3 = bb @ fc_w1.T + fc_b1
    fc_w2 = np.asarray(p["fc_w2"], np.float64)
    fc_b2 = np.asarray(p["fc_b2"], np.float64)
    fc_w3 = np.asarray(p["fc_w3"], np.float64)
    fc_b3 = np.asarray(p["fc_b3"], np.float64)
    W4 = np.zeros((64, 17)); W4[:, :16] = fc_w2.T
    b4 = np.zeros((17,)); b4[:16] = fc_b2; b4[16] = 1.0
    W5 = np.zeros((17, 1)); W5[:16, 0] = fc_w3[0]; W5[16, 0] = fc_b3[0]

    # Vector-assigned tiles use relu + decomposition:
    #   Lrelu(v) = 0.01*v + 0.99*relu(v)
    # so their W2 rows are scaled by 0.99 and the 0.01*v path (linear in x)
    # folds into W12 / b2f.  Scalar-assigned tiles compute Lrelu exactly on
    # the ACT engine, so their W2 rows stay unscaled and contribute nothing
    # to the fold.
    vmask = np.zeros((NF, 1))
    for t in range(KT):
        if t in VEC_TILES:
            vmask[128 * t:128 * (t + 1)] = 1.0
    W2s = np.where(vmask > 0, (1.0 - LRELU_NEG) * W2p, W2p)
    W12 = LRELU_NEG * ((W1p * vmask[:, 0][None, :]) @ W2p)
    b2f = LRELU_NEG * ((c1p * vmask[:, 0]) @ W2p) + b2p

    f32 = np.float32
    f16 = np.float16
    dev = {}
    # step-1 weights packed for row-pairing: slot s holds M-tiles 2s | 2s+1
    w1 = np.zeros((128, N_S1, 128), f16)
    for s in range(N_S1):
        w1[0:64, s, :] = W1p[:, 128 * (2 * s):128 * (2 * s + 1)]
        if 2 * s + 1 < KT:
            w1[64:128, s, :] = W1p[:, 128 * (2 * s + 1):128 * (2 * s + 2)]
    dev["w1"] = w1
    c1t = np.zeros((128, KT), f32)
    for t in range(KT):
        c1t[:, t] = c1p[128 * t:128 * (t + 1)]
    dev["c1t"] = c1t
    w2 = np.zeros((128, KT, 128), f16)
    for t in range(KT):
        w2[:, t, :] = W2s[128 * t:128 * (t + 1), :]
    dev["w2"] = w2
    dev["w12"] = W12.astype(f16)
    dev["b2f"] = b2f.reshape(128, 1).astype(f32)
    dev["w3"] = W3.astype(f16)
    dev["b3"] = b3.reshape(64, 1).astype(f32)
    dev["w4"] = W4.astype(f16)
    dev["b4"] = b4.reshape(17, 1).astype(f32)
    dev["w5"] = W5.astype(f16)
    return dev


# ---------------------------------------------------------------- device IR
def _build_nc(n_pairs=N_PAIRS):
    import concourse.mybir as mybir
    import concourse.tile as tile
    from concourse import bacc
    from contextlib import ExitStack

    dt = mybir.dt
    AF = mybir.ActivationFunctionType
    ALU = mybir.AluOpType
    f32 = dt.float32
    f16 = dt.float16
    bc = n_pairs * PAIR_N

    nc = bacc.Bacc("TRN2", target_bir_lowering=False, debug=False,
                   num_devices=N_CORES)

    xx_d = nc.dram_tensor("xx", [128, bc], f16, kind="ExternalInput")
    w1_d = nc.dram_tensor("w1", [128, N_S1, 128], f16, kind="ExternalInput")
    c1t_d = nc.dram_tensor("c1t", [128, KT], f32, kind="ExternalInput")
    w2_d = nc.dram_tensor("w2", [128, KT, 128], f16, kind="ExternalInput")
    w12_d = nc.dram_tensor("w12", [64, 128], f16, kind="ExternalInput")
    b2f_d = nc.dram_tensor("b2f", [128, 1], f32, kind="ExternalInput")
    w3_d = nc.dram_tensor("w3", [128, 64], f16, kind="ExternalInput")
    b3_d = nc.dram_tensor("b3", [64, 1], f32, kind="ExternalInput")
    w4_d = nc.dram_tensor("w4", [64, 17], f16, kind="ExternalInput")
    b4_d = nc.dram_tensor("b4", [17, 1], f32, kind="ExternalInput")
    w5_d = nc.dram_tensor("w5", [17, 1], f16, kind="ExternalInput")
    o_d = nc.dram_tensor("o", [1, bc], f32, kind="ExternalOutput")

    with tile.TileContext(nc) as tc, ExitStack() as ctx:
        wpool = ctx.enter_context(tc.tile_pool(name="wpool", bufs=1))
        xpool = ctx.enter_context(tc.tile_pool(name="xpool", bufs=3))
        ypool = ctx.enter_context(tc.tile_pool(name="ypool", bufs=KT + 3))
        spool = ctx.enter_context(tc.tile_pool(name="spool", bufs=2))
        ps1p = ctx.enter_context(tc.tile_pool(name="ps1p", bufs=3, space="PSUM"))
        ps2p = ctx.enter_context(tc.tile_pool(name="ps2p", bufs=1, space="PSUM"))

        # pair-0 input first so compute can start while the rest streams in
        xx_first = xpool.tile([128, PAIR_N], f16, tag="xx", name="xx_first")
        nc.sync.dma_start(xx_first[:], xx_d[:, 0:PAIR_N])
        # weight DMAs spread across engine queues: gpsimd carries the
        # step-1-critical tensors, vector the (later-needed) step-2 weights,
        # scalar the fold + tail weights.
        w1_t = wpool.tile([128, N_S1, 128], f16)
        nc.gpsimd.dma_start(w1_t[:], w1_d[:])
        c1t_t = wpool.tile([128, KT], f32)
        nc.gpsimd.dma_start(c1t_t[:], c1t_d[:])
        b2f_t = wpool.tile([128, 1], f32)
        nc.gpsimd.dma_start(b2f_t[:], b2f_d[:])
        w2_t = wpool.tile([128, KT, 128], f16)
        nc.scalar.dma_start(w2_t[:], w2_d[:])
        w12_t = wpool.tile([64, 128], f16)
        nc.gpsimd.dma_start(w12_t[:], w12_d[:])
        w3_t = wpool.tile([128, 64], f16)
        nc.gpsimd.dma_start(w3_t[:], w3_d[:])
        b3_t = wpool.tile([64, 1], f32)
        nc.gpsimd.dma_start(b3_t[:], b3_d[:])
        w4_t = wpool.tile([64, 17], f16)
        nc.gpsimd.dma_start(w4_t[:], w4_d[:])
        b4_t = wpool.tile([17, 1], f32)
        nc.gpsimd.dma_start(b4_t[:], b4_d[:])
        w5_t = wpool.tile([17, 1], f16)
        nc.gpsimd.dma_start(w5_t[:], w5_d[:])

        # PE warm-up: ~4us of dummy back-to-back matmuls on a zeroed tile so
        # the HAM clock-gate opens (1.2 -> 2.4 GHz) during the startup DMA
        # window instead of half-way through pair 1.
        wu_t = wpool.tile([64, 640], f16, name="wu")
        nc.vector.memset(wu_t[:], 0.0)
        wups = ps1p.tile([128, CHUNK], f32, tag="ps1", name="wups")
        for i in range(9):
            nc.tensor.matmul(wups[:], wu_t[:, 0:128], wu_t[:, 128:640],
                             start=True, stop=True)

        def make_tail_stages(p, ps2):
            """The per-pair serial tail (h -> g1 -> g2 -> out).  The h ACT
            pops at slot 0 of the next pair (it is the Scalar queue's first
            op there); the matmul + later ACT stages run as fillers inside
            the next pair's chain phase, where the PE is already
            row-conflicted, so they never poison the pure slot stream."""
            st = {}

            def s0():
                st["h"] = spool.tile([128, PAIR_N], f16, tag="h", name=f"h_{p}")
                nc.scalar.activation(st["h"][:], ps2[:], AF.Lrelu,
                                     bias=b2f_t[:, 0:1], alpha=LRELU_NEG)

            def s1():
                st["g1ps"] = ps1p.tile([64, PAIR_N], f32, tag="ps1",
                                       name=f"g1ps_{p}")
                for h in range(2):
                    sl = slice(h * CHUNK, (h + 1) * CHUNK)
                    nc.tensor.matmul(st["g1ps"][:, sl], w3_t[:],
                                     st["h"][:, sl], start=True, stop=True)

            def s2():
                st["g1"] = spool.tile([64, PAIR_N], f16, tag="g1",
                                      name=f"g1_{p}")
                nc.scalar.activation(st["g1"][:], st["g1ps"][:], AF.Lrelu,
                                     bias=b3_t[:, 0:1], alpha=LRELU_NEG)

            def s3():
                st["g2ps"] = ps1p.tile([17, PAIR_N], f32, tag="ps1",
                                       name=f"g2ps_{p}")
                for h in range(2):
                    sl = slice(h * CHUNK, (h + 1) * CHUNK)
                    nc.tensor.matmul(st["g2ps"][:, sl], w4_t[:],
                                     st["g1"][:, sl], start=True, stop=True)

            def s4():
                st["g2"] = spool.tile([17, PAIR_N], f16, tag="g2",
                                      name=f"g2_{p}")
                nc.scalar.activation(st["g2"][:], st["g2ps"][:], AF.Lrelu,
                                     bias=b4_t[:, 0:1], alpha=LRELU_NEG)

            def s5():
                st["ops"] = ps1p.tile([1, PAIR_N], f32, tag="ps1",
                                      name=f"ops_{p}")
                for h in range(2):
                    sl = slice(h * CHUNK, (h + 1) * CHUNK)
                    nc.tensor.matmul(st["ops"][:, sl], w5_t[:],
                                     st["g2"][:, sl], start=True, stop=True)

            def s6():
                o_t = spool.tile([1, PAIR_N], f32, tag="o", name=f"o_{p}")
                nc.vector.tensor_copy(o_t[:], st["ops"][:])
                nc.sync.dma_start(o_d[:, p * PAIR_N:(p + 1) * PAIR_N], o_t[:])

            return [s0, s1, s2, s3, s4, s5, s6]

        tail_stages = []

        xx_next = xx_first
        for p in range(n_pairs):
            xx_t = xx_next
            stages = tail_stages
            tail_stages = []

            ps2 = ps2p.tile([128, PAIR_N], f32, tag="ps2", name=f"ps2_{p}")
            ytiles = [None] * KT
            chain_len = [t1 - t0 for t0, t1 in GROUP_TR]
            chain_pos = [0] * 4

            def emit_chain_rounds(lag_tile, max_rounds):
                """Emit up to max_rounds chunk-major rounds of ready chain
                steps: all ready groups' h0 chunks back-to-back, then their
                h1 chunks, so up to four 32-column tile positions stream
                concurrently."""
                for _ in range(max_rounds):
                    rg = [g for g in range(4)
                          if chain_pos[g] < chain_len[g]
                          and GROUP_TR[g][0] + chain_pos[g] <= lag_tile]
                    if not rg:
                        return
                    for h in range(2):
                        sl = slice(h * CHUNK, (h + 1) * CHUNK)
                        for g in rg:
                            t = GROUP_TR[g][0] + chain_pos[g]
                            nc.tensor.matmul(
                                ps2[32 * g:32 * (g + 1), sl],
                                w2_t[:, t, 32 * g:32 * (g + 1)],
                                ytiles[t][:, sl], start=False,
                                stop=(chain_pos[g] == chain_len[g] - 1),
                                tile_position=(0, 32 * g),
                                skip_group_check=True)
                    for g in rg:
                        chain_pos[g] += 1

            # ---- slot phase: [A0, B0, (A0, B0 filler dup), A1, B1] per
            # slot, with ready chain-step rounds interleaved BETWEEN slots.
            # The slot stream is evacuation-paced; the interleaved chain
            # work plus the h0 duplicate (overwritten by the second
            # start=True, so results are identical) keep the PE dense enough
            # that the HAM clock-gate stays at K=8/8 (idle gaps would
            # re-throttle the PE to 1.2 GHz every pair).
            for s in range(N_S1):
                if s == 0 and stages:
                    stages[0]()          # h-act of prev pair (Scalar op #1)
                if p + 1 < n_pairs and s == 1:
                    xx_next = xpool.tile([128, PAIR_N], f16, tag="xx",
                                         name=f"xx_{p + 1}")
                    nc.sync.dma_start(
                        xx_next[:],
                        xx_d[:, (p + 1) * PAIR_N:(p + 2) * PAIR_N])

                tA, tB = 2 * s, 2 * s + 1
                psA = ps1p.tile([128, PAIR_N], f32, tag="ps1",
                                name=f"psA_{p}_{s}")
                psB = None
                if tB < KT:
                    psB = ps1p.tile([128, PAIR_N], f32, tag="ps1",
                                    name=f"psB_{p}_{s}")
                for h in range(2):
                    sl = slice(h * CHUNK, (h + 1) * CHUNK)
                    for _dup in range(2 if h == 0 else 1):
                        nc.tensor.matmul(
                            psA[:, sl], w1_t[0:64, s, :], xx_t[0:64, sl],
                            start=True, stop=True, tile_position=(0, 0))
                        if psB is not None:
                            nc.tensor.matmul(
                                psB[:, sl], w1_t[64:128, s, :],
                                xx_t[64:128, sl],
                                start=True, stop=True, tile_position=(64, 0))
                for t, ps in ((tA, psA),) + (((tB, psB),) if psB is not None
                                             else ()):
                    y_t = ypool.tile([128, PAIR_N], f16, tag="y",
                                     name=f"y_{p}_{t}")
                    if t in VEC_TILES:
                        nc.vector.tensor_scalar(
                            y_t[:], ps[:], c1t_t[:, t:t + 1], 0.0,
                            ALU.add, ALU.max)
                    else:
                        nc.scalar.activation(
                            y_t[:], ps[:], AF.Lrelu, bias=c1t_t[:, t:t + 1],
                            alpha=LRELU_NEG)
                    ytiles[t] = y_t

                if s == 2:
                    # first write of ps2: the 0.01*y fold path (vector tiles
                    # only).  Emitted late enough that the previous pair's
                    # h-act has released the bank.
                    for h in range(2):
                        sl = slice(h * CHUNK, (h + 1) * CHUNK)
                        nc.tensor.matmul(
                            ps2[:, sl], w12_t[:], xx_t[0:64, sl],
                            start=True, stop=False, tile_position=(0, 0),
                            skip_group_check=True)
                if s >= 3:
                    emit_chain_rounds(2 * (s - 2) + 1, 1)

            # ---- flush remaining chain steps + previous pair's tail stages
            # as fillers between rounds.
            fillers = stages[1:] if stages else []
            fill_i = 0
            while any(chain_pos[g] < chain_len[g] for g in range(4)):
                emit_chain_rounds(KT, 1)
                if fill_i < len(fillers):
                    fillers[fill_i]()
                    fill_i += 1
            while fill_i < len(fillers):
                fillers[fill_i]()
                fill_i += 1

            tail_stages = make_tail_stages(p, ps2)

        for st in tail_stages:
            st()

    nc.compile()
    return nc


# ---------------------------------------------------------------- execution
_NC_CACHE = {}
LAST_RESULT = None


def _prep_inputs(inputs):
    board = np.ascontiguousarray(np.asarray(inputs["board"], np.float32))
    x = board.reshape(B_TOTAL, 64)
    dev = _fold_params(inputs)
    in_maps = []
    for c in range(N_CORES):
        xc = np.ascontiguousarray(x[c * BC:(c + 1) * BC].T)      # [64, BC]
        m = dict(dev)
        m["xx"] = np.ascontiguousarray(
            np.vstack([xc, xc]).astype(np.float16))              # [128, BC]
        in_maps.append(m)
    return in_maps


def kernel(**inputs):
    global LAST_RESULT
    from concourse.bass_utils import run_bass_kernel_spmd

    if "nc" not in _NC_CACHE:
        _NC_CACHE["nc"] = _build_nc()
    nc = _NC_CACHE["nc"]

    in_maps = _prep_inputs(inputs)
    res = run_bass_kernel_spmd(nc, in_maps, core_ids=list(range(N_CORES)))
    LAST_RESULT = res
    out = np.concatenate([r["o"].reshape(-1) for r in res.results])
    return out.reshape(B_TOTAL, 1).astype(np.float32)


# revision 11
# speedup vs baseline: 1.4672x; 1.2851x over previous
"""Trainium2 Bass kernel for nn_BlockBlastValueNet1PmultikernelFlattenned.

Strategy (v2)
-------------
The network is 8 tiny conv branches over an 8x8 board followed by small MLPs.
Because the board has only 64 pixels, every conv branch (pad const 1.0 +
valid conv + bias) is an affine map of the 64 board values.  The whole net
folds into:

    y  = x @ W1 + c1                     # [B, NF]  (NF = 2944 padded)
    h  = Lrelu( Lrelu(y) @ W2' + b2 )    # per-branch first FC, block diagonal
    g1 = Lrelu( h @ W3 + b3 )            # branch second FC fused with fc1
    g2 = Lrelu( g1 @ W4 + b4 )           # fc2 (augmented with a ones column)
    out = g2 @ W5                        # fc3 (bias folded via augmentation)

Data-parallel over 8 NeuronCores (batch 65536 -> 8192/core), processed in
8 pair-iterations of 1024 samples.  Features live on SBUF partitions,
samples stream on the free dim.  Matmuls run in fp16 (full PE rate, warm
2.4 GHz clock).

The binding resource is PSUM evacuation: every y element must pass through
the Scalar or Vector engine exactly once (1 elem/cycle/lane each), so the
whole schedule is built to keep both engines 100% busy while the PE (which
has slack) works around them:

* Step-1 (x -> y) is row-packed: two 64x128 weight tiles at tile_position
  (0,0)/(64,0) run concurrently, 12 slots per pair.
* y evacuation is split ~12/11 between Vector (tensor_scalar add+max =
  relu(y+c1), with the 0.01*y leaky path folded into a small 64->128
  "fold" matmul W12 covering only the vector-assigned tiles) and Scalar
  (exact Lrelu via ACT with per-partition bias — no decomposition needed).
* Step-2 (y -> h) exploits W2's block-diagonal structure with PE column
  tiling: branches are paired into four 32-column groups; each group is an
  accumulation chain over only ITS y tiles at tile_position (0, 32g), so
  the four chains (9/6/5/6 tiles) run concurrently in the four column
  quadrants.  This removes the old row-half partial-sum merge entirely
  (saving two 128x1024 Vector ops per pair) and halves step-2 PE time.
* The serial tail (h -> g1 -> g2 -> out) runs as fp16 matmuls + Scalar
  ACTs, interleaved two slots apart into the NEXT pair's slot stream so
  its cross-engine latencies never head-of-line-block the PE queue.
* ps2 (the h accumulator) is written first by the fold matmul at slot 2,
  after the previous pair's h evacuation has had time to finish (avoids
  the PE queue parking on the PSUM WAR dependency).
* Startup weight DMAs are spread over the gpsimd/vector/scalar queues so
  the first matmul starts ~3us in instead of ~14us.
"""

import numpy as np

# ---------------------------------------------------------------- constants
SPECS = [(1, 1, 1, 0, 0), (2, 2, 6, 1, 1), (3, 3, 8, 1, 1), (4, 4, 8, 2, 2),
         (5, 5, 16, 2, 2), (8, 8, 32, 0, 0), (1, 8, 4, 0, 0), (8, 1, 4, 0, 0)]
BOARD = 8
B_TOTAL = 65536
N_CORES = 8
BC = B_TOTAL // N_CORES          # 8192 samples per core
PAIR_N = 1024                    # samples per pair-iteration
CHUNK = 512                      # matmul moving width (1 psum bank fp32)
N_PAIRS = BC // PAIR_N           # 8

# column-group pairing of branches (2 branches x 16 h-outputs = 32 cols each).
# y rows are packed tightly in group order; a K-tile straddling a group
# boundary appears in BOTH groups' chains (the block-diagonal W2 already has
# zeros for the other group's columns).
GROUPS = [[4, 5], [3, 6], [2, 7], [1, 0]]
_BR_N = []
for kh, kw, fs, ph, pw in SPECS:
    _BR_N.append((BOARD + 2 * ph - kh + 1) * (BOARD + 2 * pw - kw + 1) * fs)
_NF_TRUE = sum(_BR_N)            # 2830
KT = -(-_NF_TRUE // 128)         # 23 K-tiles / M-tiles of y
NF = KT * 128                    # 2944 (last tile zero-padded)
_G_ROWS = []
_o = 0
for g in GROUPS:
    sz = sum(_BR_N[b] for b in g)
    _G_ROWS.append((_o, _o + sz))
    _o += sz
GROUP_TR = [(r0 // 128, -(-r1 // 128)) for r0, r1 in _G_ROWS]  # tile ranges
N_S1 = (KT + 1) // 2             # step-1 row-packed slots (12)

LRELU_NEG = 0.01

# evac engine split: these tiles go to the Vector engine (relu + fold path),
# the rest to Scalar (exact Lrelu ACT).  Slots 0/1 are all-vector because at
# pair start the Scalar engine is still busy with the previous pair's tail.
VEC_TILES = frozenset({0, 1, 2, 3, 5, 7, 9, 11, 13, 15, 17, 19})


# ---------------------------------------------------------------- host fold
def _fold_params(p):
    """Fold conv branches + MLPs into the dense pipeline weights (float64)."""
    n_of = _BR_N
    W1_of, c1_of = {}, {}
    for i, (kh, kw, fs, ph, pw) in enumerate(SPECS):
        Ho = BOARD + 2 * ph - kh + 1
        Wo = BOARD + 2 * pw - kw + 1
        cw = np.asarray(p[f"b{i}_cw"], np.float64)
        cb = np.asarray(p[f"b{i}_cb"], np.float64)
        W1 = np.zeros((64, n_of[i]))
        c1 = np.zeros((n_of[i],))
        for f in range(fs):
            for oh in range(Ho):
                for ow in range(Wo):
                    oi = (f * Ho + oh) * Wo + ow
                    c1[oi] += cb[f]
                    for u in range(kh):
                        for v in range(kw):
                            r, c = oh + u - ph, ow + v - pw
                            w = cw[f, 0, u, v]
                            if 0 <= r < 8 and 0 <= c < 8:
                                W1[r * 8 + c, oi] += w
                            else:
                                c1[oi] += w        # pad value is 1.0
        W1_of[i] = W1
        c1_of[i] = c1

    # K-layout: groups concatenated tightly; only the final tile is padded
    K_start = {}
    off = 0
    for g in GROUPS:
        for b in g:
            K_start[b] = off
            off += n_of[b]
    assert off == _NF_TRUE
    border = [b for g in GROUPS for b in g]       # h block order
    hpos = {b: j * 16 for j, b in enumerate(border)}

    W1p = np.zeros((64, NF))
    c1p = np.zeros((NF,))
    W2p = np.zeros((NF, 128))
    b2p = np.zeros((128,))
    for b in range(8):
        s, n, hp = K_start[b], n_of[b], hpos[b]
        W1p[:, s:s + n] = W1_of[b]
        c1p[s:s + n] = c1_of[b]
        W2p[s:s + n, hp:hp + 16] = np.asarray(p[f"b{b}_w1"], np.float64).T
        b2p[hp:hp + 16] = np.asarray(p[f"b{b}_b1"], np.float64)

    Wb = np.zeros((128, 64))
    bb = np.zeros((64,))
    for b in range(8):
        hp = hpos[b]
        Wb[hp:hp + 16, 8 * b:8 * b + 8] = np.asarray(p[f"b{b}_w2"], np.float64).T
        bb[8 * b:8 * b + 8] = np.asarray(p[f"b{b}_b2"], np.float64)
    fc_w1 = np.asarray(p["fc_w1"], np.float64)
    fc_b1 = np.asarray(p["fc_b1"], np.float64)
    W3 = Wb @ fc_w1.T
    b3 = bb @ fc_w1.T + fc_b1
    fc_w2 = np.asarray(p["fc_w2"], np.float64)
    fc_b2 = np.asarray(p["fc_b2"], np.float64)
    fc_w3 = np.asarray(p["fc_w3"], np.float64)
    fc_b3 = np.asarray(p["fc_b3"], np.float64)
    W4 = np.zeros((64, 17)); W4[:, :16] = fc_w2.T
    b4 = np.zeros((17,)); b4[:16] = fc_b2; b4[16] = 1.0
    W5 = np.zeros((17, 1)); W5[:16, 0] = fc_w3[0]; W5[16, 0] = fc_b3[0]

    # Vector-assigned tiles use relu + decomposition:
    #   Lrelu(v) = 0.01*v + 0.99*relu(v)
    # so their W2 rows are scaled by 0.99 and the 0.01*v path (linear in x)
    # folds into W12 / b2f.  Scalar-assigned tiles compute Lrelu exactly on
    # the ACT engine, so their W2 rows stay unscaled and contribute nothing
    # to the fold.
    vmask = np.zeros((NF, 1))
    for t in range(KT):
        if t in VEC_TILES:
            vmask[128 * t:128 * (t + 1)] = 1.0
    W2s = np.where(vmask > 0, (1.0 - LRELU_NEG) * W2p, W2p)
    W12 = LRELU_NEG * ((W1p * vmask[:, 0][None, :]) @ W2p)
    b2f = LRELU_NEG * ((c1p * vmask[:, 0]) @ W2p) + b2p

    f32 = np.float32
    f16 = np.float16
    dev = {}
    # step-1 weights packed for row-pairing: slot s holds M-tiles 2s | 2s+1
    w1 = np.zeros((128, N_S1, 128), f16)
    for s in range(N_S1):
        w1[0:64, s, :] = W1p[:, 128 * (2 * s):128 * (2 * s + 1)]
        if 2 * s + 1 < KT:
            w1[64:128, s, :] = W1p[:, 128 * (2 * s + 1):128 * (2 * s + 2)]
    dev["w1"] = w1
    c1t = np.zeros((128, KT), f32)
    for t in range(KT):
        c1t[:, t] = c1p[128 * t:128 * (t + 1)]
    dev["c1t"] = c1t
    w2 = np.zeros((128, KT, 128), f16)
    for t in range(KT):
        w2[:, t, :] = W2s[128 * t:128 * (t + 1), :]
    dev["w2"] = w2
    dev["w12"] = W12.astype(f16)
    dev["b2f"] = b2f.reshape(128, 1).astype(f32)
    dev["w3"] = W3.astype(f16)
    dev["b3"] = b3.reshape(64, 1).astype(f32)
    dev["w4"] = W4.astype(f16)
    dev["b4"] = b4.reshape(17, 1).astype(f32)
    dev["w5"] = W5.astype(f16)
    return dev


# ---------------------------------------------------------------- device IR
def _build_nc(n_pairs=N_PAIRS):
    import concourse.mybir as mybir
    import concourse.tile as tile
    from concourse import bacc
    from contextlib import ExitStack

    dt = mybir.dt
    AF = mybir.ActivationFunctionType
    ALU = mybir.AluOpType
    f32 = dt.float32
    f16 = dt.float16
    bc = n_pairs * PAIR_N

    nc = bacc.Bacc("TRN2", target_bir_lowering=False, debug=False,
                   num_devices=N_CORES)

    xx_d = nc.dram_tensor("xx", [128, bc], f16, kind="ExternalInput")
    w1_d = nc.dram_tensor("w1", [128, N_S1, 128], f16, kind="ExternalInput")
    c1t_d = nc.dram_tensor("c1t", [128, KT], f32, kind="ExternalInput")
    w2_d = nc.dram_tensor("w2", [128, KT, 128], f16, kind="ExternalInput")
    w12_d = nc.dram_tensor("w12", [64, 128], f16, kind="ExternalInput")
    b2f_d = nc.dram_tensor("b2f", [128, 1], f32, kind="ExternalInput")
    w3_d = nc.dram_tensor("w3", [128, 64], f16, kind="ExternalInput")
    b3_d = nc.dram_tensor("b3", [64, 1], f32, kind="ExternalInput")
    w4_d = nc.dram_tensor("w4", [64, 17], f16, kind="ExternalInput")
    b4_d = nc.dram_tensor("b4", [17, 1], f32, kind="ExternalInput")
    w5_d = nc.dram_tensor("w5", [17, 1], f16, kind="ExternalInput")
    o_d = nc.dram_tensor("o", [1, bc], f32, kind="ExternalOutput")

    with tile.TileContext(nc) as tc, ExitStack() as ctx:
        wpool = ctx.enter_context(tc.tile_pool(name="wpool", bufs=1))
        xpool = ctx.enter_context(tc.tile_pool(name="xpool", bufs=3))
        ypool = ctx.enter_context(tc.tile_pool(name="ypool", bufs=KT + 3))
        spool = ctx.enter_context(tc.tile_pool(name="spool", bufs=2))
        ps1p = ctx.enter_context(tc.tile_pool(name="ps1p", bufs=3, space="PSUM"))
        ps2p = ctx.enter_context(tc.tile_pool(name="ps2p", bufs=1, space="PSUM"))

        # pair-0 input first so compute can start while the rest streams in
        xx_first = xpool.tile([128, PAIR_N], f16, tag="xx", name="xx_first")
        nc.sync.dma_start(xx_first[:], xx_d[:, 0:PAIR_N])
        # weight DMAs spread across engine queues: gpsimd carries the
        # step-1-critical tensors, vector the (later-needed) step-2 weights,
        # scalar the fold + tail weights.
        w1_t = wpool.tile([128, N_S1, 128], f16)
        nc.gpsimd.dma_start(w1_t[:], w1_d[:])
        c1t_t = wpool.tile([128, KT], f32)
        nc.gpsimd.dma_start(c1t_t[:], c1t_d[:])
        b2f_t = wpool.tile([128, 1], f32)
        nc.gpsimd.dma_start(b2f_t[:], b2f_d[:])
        w2_t = wpool.tile([128, KT, 128], f16)
        nc.scalar.dma_start(w2_t[:], w2_d[:])
        w12_t = wpool.tile([64, 128], f16)
        nc.gpsimd.dma_start(w12_t[:], w12_d[:])
        w3_t = wpool.tile([128, 64], f16)
        nc.gpsimd.dma_start(w3_t[:], w3_d[:])
        b3_t = wpool.tile([64, 1], f32)
        nc.gpsimd.dma_start(b3_t[:], b3_d[:])
        w4_t = wpool.tile([64, 17], f16)
        nc.gpsimd.dma_start(w4_t[:], w4_d[:])
        b4_t = wpool.tile([17, 1], f32)
        nc.gpsimd.dma_start(b4_t[:], b4_d[:])
        w5_t = wpool.tile([17, 1], f16)
        nc.gpsimd.dma_start(w5_t[:], w5_d[:])

        # PE warm-up: ~4us of dummy back-to-back matmuls on a zeroed tile so
        # the HAM clock-gate opens (1.2 -> 2.4 GHz) during the startup DMA
        # window instead of half-way through pair 1.
        wu_t = wpool.tile([64, 640], f16, name="wu")
        nc.vector.memset(wu_t[:], 0.0)
        wups = ps1p.tile([128, CHUNK], f32, tag="ps1", name="wups")
        for i in range(9):
            nc.tensor.matmul(wups[:], wu_t[:, 0:128], wu_t[:, 128:640],
                             start=True, stop=True)

        def make_tail_stages(p, ps2):
            """The per-pair serial tail (h -> g1 -> g2 -> out).  The h ACT
            pops at slot 0 of the next pair (it is the Scalar queue's first
            op there); the matmul + later ACT stages run as fillers inside
            the next pair's chain phase, where the PE is already
            row-conflicted, so they never poison the pure slot stream."""
            st = {}

            def s0():
                st["h"] = spool.tile([128, PAIR_N], f16, tag="h", name=f"h_{p}")
                nc.scalar.activation(st["h"][:], ps2[:], AF.Lrelu,
                                     bias=b2f_t[:, 0:1], alpha=LRELU_NEG)

            def s1():
                st["g1ps"] = ps1p.tile([64, PAIR_N], f32, tag="ps1",
                                       name=f"g1ps_{p}")
                for h in range(2):
                    sl = slice(h * CHUNK, (h + 1) * CHUNK)
                    nc.tensor.matmul(st["g1ps"][:, sl], w3_t[:],
                                     st["h"][:, sl], start=True, stop=True)

            def s2():
                st["g1"] = spool.tile([64, PAIR_N], f16, tag="g1",
                                      name=f"g1_{p}")
                nc.scalar.activation(st["g1"][:], st["g1ps"][:], AF.Lrelu,
                                     bias=b3_t[:, 0:1], alpha=LRELU_NEG)

            def s3():
                st["g2ps"] = ps1p.tile([17, PAIR_N], f32, tag="ps1",
                                       name=f"g2ps_{p}")
                for h in range(2):
                    sl = slice(h * CHUNK, (h + 1) * CHUNK)
                    nc.tensor.matmul(st["g2ps"][:, sl], w4_t[:],
                                     st["g1"][:, sl], start=True, stop=True)

            def s4():
                st["g2"] = spool.tile([17, PAIR_N], f16, tag="g2",
                                      name=f"g2_{p}")
                nc.scalar.activation(st["g2"][:], st["g2ps"][:], AF.Lrelu,
                                     bias=b4_t[:, 0:1], alpha=LRELU_NEG)

            def s5():
                st["ops"] = ps1p.tile([1, PAIR_N], f32, tag="ps1",
                                      name=f"ops_{p}")
                for h in range(2):
                    sl = slice(h * CHUNK, (h + 1) * CHUNK)
                    nc.tensor.matmul(st["ops"][:, sl], w5_t[:],
                                     st["g2"][:, sl], start=True, stop=True)

            def s6():
                o_t = spool.tile([1, PAIR_N], f32, tag="o", name=f"o_{p}")
                nc.vector.tensor_copy(o_t[:], st["ops"][:])
                nc.sync.dma_start(o_d[:, p * PAIR_N:(p + 1) * PAIR_N], o_t[:])

            return [s0, s1, s2, s3, s4, s5, s6]

        tail_stages = []
        # tail stage -> slot index it pops at (spaced 2 apart)
        TAIL_SLOT = {0: 0, 1: 2, 2: 4, 3: 6, 4: 8, 5: 10, 6: 11}

        xx_next = xx_first
        for p in range(n_pairs):
            xx_t = xx_next
            stages = tail_stages
            tail_stages = []

            ps2 = ps2p.tile([128, PAIR_N], f32, tag="ps2", name=f"ps2_{p}")
            ytiles = [None] * KT
            chain_len = [t1 - t0 for t0, t1 in GROUP_TR]
            chain_pos = [0] * 4

            def emit_chain_rounds(lag_tile, max_rounds):
                """Emit up to max_rounds chunk-major rounds of ready chain
                steps: all ready groups' h0 chunks back-to-back, then their
                h1 chunks, so up to four 32-column tile positions stream
                concurrently."""
                for _ in range(max_rounds):
                    rg = [g for g in range(4)
                          if chain_pos[g] < chain_len[g]
                          and GROUP_TR[g][0] + chain_pos[g] <= lag_tile]
                    if not rg:
                        return
                    for h in range(2):
                        sl = slice(h * CHUNK, (h + 1) * CHUNK)
                        for g in rg:
                            t = GROUP_TR[g][0] + chain_pos[g]
                            nc.tensor.matmul(
                                ps2[32 * g:32 * (g + 1), sl],
                                w2_t[:, t, 32 * g:32 * (g + 1)],
                                ytiles[t][:, sl], start=False,
                                stop=(chain_pos[g] == chain_len[g] - 1),
                                tile_position=(0, 32 * g),
                                skip_group_check=True)
                    for g in rg:
                        chain_pos[g] += 1

            # ---- slot phase: [A0, B0, A1, B1] per slot, with the previous
            # pair's tail stages and ready chain-step rounds interleaved
            # between slots.  Keeping the K=128 chain matmuls sprinkled
            # through the K=64 slot stream is what keeps the PE HAM
            # activity monitor above its un-throttle threshold — phases of
            # pure K=64 slot matmuls measure as half-active and get clocked
            # down to 1.2 GHz even when they are back-to-back.
            stage_i = 0
            for s in range(N_S1):
                while (stage_i < len(stages)
                       and TAIL_SLOT[stage_i] <= s):
                    stages[stage_i]()
                    stage_i += 1
                if p + 1 < n_pairs and s == 1:
                    xx_next = xpool.tile([128, PAIR_N], f16, tag="xx",
                                         name=f"xx_{p + 1}")
                    nc.sync.dma_start(
                        xx_next[:],
                        xx_d[:, (p + 1) * PAIR_N:(p + 2) * PAIR_N])

                tA, tB = 2 * s, 2 * s + 1
                psA = ps1p.tile([128, PAIR_N], f32, tag="ps1",
                                name=f"psA_{p}_{s}")
                psB = None
                if tB < KT:
                    psB = ps1p.tile([128, PAIR_N], f32, tag="ps1",
                                    name=f"psB_{p}_{s}")
                for h in range(2):
                    sl = slice(h * CHUNK, (h + 1) * CHUNK)
                    nc.tensor.matmul(
                        psA[:, sl], w1_t[0:64, s, :], xx_t[0:64, sl],
                        start=True, stop=True, tile_position=(0, 0))
                    if psB is not None:
                        nc.tensor.matmul(
                            psB[:, sl], w1_t[64:128, s, :],
                            xx_t[64:128, sl],
                            start=True, stop=True, tile_position=(64, 0))
                for t, ps in ((tA, psA),) + (((tB, psB),) if psB is not None
                                             else ()):
                    y_t = ypool.tile([128, PAIR_N], f16, tag="y",
                                     name=f"y_{p}_{t}")
                    if t in VEC_TILES:
                        nc.vector.tensor_scalar(
                            y_t[:], ps[:], c1t_t[:, t:t + 1], 0.0,
                            ALU.add, ALU.max)
                    else:
                        nc.scalar.activation(
                            y_t[:], ps[:], AF.Lrelu, bias=c1t_t[:, t:t + 1],
                            alpha=LRELU_NEG)
                    ytiles[t] = y_t

                if s == 2:
                    # first write of ps2: the 0.01*y fold path (vector tiles
                    # only).  Emitted late enough that the previous pair's
                    # h-act has released the bank.
                    for h in range(2):
                        sl = slice(h * CHUNK, (h + 1) * CHUNK)
                        nc.tensor.matmul(
                            ps2[:, sl], w12_t[:], xx_t[0:64, sl],
                            start=True, stop=False, tile_position=(0, 0),
                            skip_group_check=True)
                if s >= 3:
                    emit_chain_rounds(2 * (s - 2) + 1, 1)

            # ---- flush remaining chain steps, then any remaining tail
            # stages of the previous pair.
            while stage_i < len(stages):
                stages[stage_i]()
                stage_i += 1
            while any(chain_pos[g] < chain_len[g] for g in range(4)):
                emit_chain_rounds(KT, 1)

            tail_stages = make_tail_stages(p, ps2)

        for st in tail_stages:
            st()

    nc.compile()
    return nc


# ---------------------------------------------------------------- execution
_NC_CACHE = {}
LAST_RESULT = None


def _prep_inputs(inputs):
    board = np.ascontiguousarray(np.asarray(inputs["board"], np.float32))
    x = board.reshape(B_TOTAL, 64)
    dev = _fold_params(inputs)
    in_maps = []
    for c in range(N_CORES):
        xc = np.ascontiguousarray(x[c * BC:(c + 1) * BC].T)      # [64, BC]
        m = dict(dev)
        m["xx"] = np.ascontiguousarray(
            np.vstack([xc, xc]).astype(np.float16))              # [128, BC]
        in_maps.append(m)
    return in_maps


def kernel(**inputs):
    global LAST_RESULT
    from concourse.bass_utils import run_bass_kernel_spmd

    if "nc" not in _NC_CACHE:
        _NC_CACHE["nc"] = _build_nc()
    nc = _NC_CACHE["nc"]

    in_maps = _prep_inputs(inputs)
    res = run_bass_kernel_spmd(nc, in_maps, core_ids=list(range(N_CORES)))
    LAST_RESULT = res
    out = np.concatenate([r["o"].reshape(-1) for r in res.results])
    return out.reshape(B_TOTAL, 1).astype(np.float32)
